# revision 4
# baseline (speedup 1.0000x reference)
"""Trainium2 Bass kernel for nn_CrossAttention (single-query cross attention).

Reference computation (B=4, C=64, H=W=128, heads h=64, dim_head d=64,
inner=4096, HW=16384):
    x[b, j, c]   = fimg[b, c, j]                       (j indexes H*W)
    q[b, h, d]   = sum_e fpsf[b, e] Wq[h*64+d, e]
    k[b, j, h, d]= sum_c x[b, j, c] Wk[h*64+d, c]
    out[b, h, j] = scale * sum_d q[b,h,d] k[b,j,h,d]

Because there is a single query per (batch, head), the attention collapses:
    W2[b, h, c]  = scale * sum_d q[b,h,d] Wk[h*64+d, c]      (tiny)
    out[b, h, j] = sum_c W2[b,h,c] fimg[b, c, j]             (one [64x64]@[64,HW]
                                                              matmul per batch)
This is a 64x FLOP reduction vs materializing k. The kernel is then
DMA-bound: each core reads its fimg shard + the (replicated) weights and
writes its output shard.

Sharding: the j (H*W = 16384) axis is split across the 8 cores (2048 each).
Every core redundantly computes W2 (it needs all heads for its output).

Device layouts (prepared host-side; host does LAYOUT only, no math):
  fpsfT  [64, 4]    : fpsf.T                       (e, b)
  WqT    [128, 2048]: [ (p%2)*64+e, (p//2)*128+m ] = Wq[128p+m, e]
                      -> lhsT slices [64(e), 128(hd)] per 128-chunk p of hd
  Wk_bd  [128, 4096]: per head-pair p: cols 128p..128p+128 hold the
                      block-diag [[Wk_{2p}[d,c], 0], [0, Wk_{2p+1}[d,c]]]
  fimg_s [256, 2048]: rows b*64+c, cols = local j
  out    [256, 2048]: rows b*64+h, cols = local j

Device compute per core:
  A: 32 matmuls  q2T[128c-chunk, 4] = WqT_chunk.T @ (fpsfT*scale)
     -> q2T psum [128, 128] with cols 4p+b
  B: 32 matmuls  w2 [128, 4] = Wk_bd_p.T @ q2T[:, 4p:4p+4]
     -> w2 psum [128, 128]: rows c + 64*(h%2), cols 4*(h//2)+b
  Assembly: per batch-pair q, block-diag lhsT bd_q [128, 128]:
     bd_q[64*half + c, 64*half + h] = W2[2q+half, h, c]
  Big: 8 matmuls [128, 512] = bd_q.T @ fimg rows-pair, copy psum->sbuf, DMA out.
"""

import sys
import types

import numpy as np

# antenv.axon_hooks is absent in this image; bass_utils imports it when
# tracing. Register a minimal stand-in before importing concourse.
if "antenv.axon_hooks" not in sys.modules:
    try:
        import antenv  # noqa: F401

        _hooks = types.ModuleType("antenv.axon_hooks")
        _hooks._hook = None

        def _set_hook(h):
            _hooks._hook = h

        _hooks.set_axon_ntff_profile_hook = _set_hook
        _hooks.get_axon_ntff_profile_hook = lambda: _hooks._hook
        sys.modules["antenv.axon_hooks"] = _hooks
        try:
            from trn_agent_boot.trn_boot import _ntff_profile_via_ctypes

            _set_hook(_ntff_profile_via_ctypes("/opt/axon/libaxon_pjrt.so"))
        except Exception:
            pass
    except ImportError:
        pass

import concourse.bass as bass  # noqa: E402
import concourse.mybir as mybir  # noqa: E402
import concourse.tile as tile  # noqa: E402
from concourse import bacc  # noqa: E402
from concourse.bass_utils import run_bass_kernel_spmd  # noqa: E402

N_CORES = 8
B, C, H, W = 4, 64, 128, 128
HEADS, DIM_HEAD = 64, 64
HW = H * W
JS = HW // N_CORES  # 2048 j-positions per core
SCALE = DIM_HEAD ** -0.5
F32 = mybir.dt.float32

_compiled = None  # cache (nc) across calls


def _build():
    nc = bacc.Bacc("TRN2", target_bir_lowering=False, debug=False,
                   num_devices=N_CORES)

    fimg_d = nc.dram_tensor("fimg_s", [2 * 128, JS], F32, kind="ExternalInput")
    fpsf_d = nc.dram_tensor("fpsfT", [64, 4], F32, kind="ExternalInput")
    wqt_d = nc.dram_tensor("WqT", [128, 2048], F32, kind="ExternalInput")
    wkbd_d = nc.dram_tensor("Wk_bd", [128, 4096], F32, kind="ExternalInput")
    out_d = nc.dram_tensor("out", [2 * 128, JS], F32, kind="ExternalOutput")

    with tile.TileContext(nc) as tc:
        with (
            tc.tile_pool(name="weights", bufs=1) as wpool,
            tc.tile_pool(name="img", bufs=1) as ipool,
            tc.tile_pool(name="small_ps", bufs=1, space="PSUM") as spsum,
            tc.tile_pool(name="big_ps", bufs=4, space="PSUM") as bpsum,
            tc.tile_pool(name="ostage", bufs=4) as opool,
        ):
            # fpsf replicated into both partition halves: matmul requires
            # lhsT and rhs to share base_partition, and odd hd-chunks of
            # WqT live at partitions 64..127.
            fpsfT = wpool.tile([128, 4], F32, tag="fpsfT")
            nc.sync.dma_start(fpsfT[0:64, :], fpsf_d.ap())
            nc.sync.dma_start(fpsfT[64:128, :], fpsf_d.ap())
            wqT = wpool.tile([128, 2048], F32, tag="wqT")
            nc.sync.dma_start(wqT[:], wqt_d.ap())
            wkbd = wpool.tile([128, 4096], F32, tag="wkbd")
            nc.sync.dma_start(wkbd[:], wkbd_d.ap())
            imgs = []
            for q in range(2):
                t = ipool.tile([128, JS], F32, tag=f"img{q}")
                nc.sync.dma_start(t[:], fimg_d.ap()[128 * q:128 * (q + 1), :])
                imgs.append(t)

            fpsf_sc = wpool.tile([128, 4], F32, tag="fpsf_sc")
            nc.scalar.mul(fpsf_sc[:], fpsfT[:], SCALE)

            # A: q2T[p_row, 4p+b] = q2[b, 128p + p_row], scaled
            q2T_ps = spsum.tile([128, 128], F32, tag="q2T_ps")
            for p in range(32):
                r, pq = p % 2, p // 2
                nc.tensor.matmul(
                    q2T_ps[:, 4 * p:4 * p + 4],
                    wqT[64 * r:64 * r + 64, 128 * pq:128 * pq + 128],
                    fpsf_sc[64 * r:64 * r + 64, :],
                    start=True, stop=True,
                )
            q2T = wpool.tile([128, 128], F32, tag="q2T")
            nc.vector.tensor_copy(q2T[:], q2T_ps[:])

            # B: w2[c + 64*(h%2), 4*(h//2)+b] = W2[b, h, c] (scaled)
            w2_ps = spsum.tile([128, 128], F32, tag="w2_ps")
            for p in range(32):
                nc.tensor.matmul(
                    w2_ps[:, 4 * p:4 * p + 4],
                    wkbd[:, 128 * p:128 * p + 128],
                    q2T[:, 4 * p:4 * p + 4],
                    start=True, stop=True,
                )

            # Assembly: bd_q[64*half + c, 64*half + h] = W2[2q+half, h, c]
            bds = []
            for q in range(2):
                bd = wpool.tile([128, 128], F32, tag=f"bd{q}")
                nc.vector.memset(bd[:], 0.0)
                for half in range(2):
                    b = 2 * q + half
                    for parity in range(2):
                        dst = bd[64 * half:64 * half + 64,
                                 64 * half + parity:64 * half + 64:2]
                        src = w2_ps[64 * parity:64 * parity + 64, b:128:4]
                        nc.vector.tensor_copy(dst, src)
                bds.append(bd)

            # Big: out rows pair q = bd_q.T @ img_q, in 512-col chunks
            for q in range(2):
                for k in range(4):
                    ps = bpsum.tile([128, 512], F32, tag="mm_ps")
                    nc.tensor.matmul(
                        ps[:], bds[q][:],
                        imgs[q][:, 512 * k:512 * k + 512],
                        start=True, stop=True,
                    )
                    ot = opool.tile([128, 512], F32, tag="ot")
                    if k % 2 == 0:
                        nc.vector.tensor_copy(ot[:], ps[:])
                    else:
                        nc.scalar.copy(ot[:], ps[:])
                    nc.sync.dma_start(
                        out_d.ap()[128 * q:128 * (q + 1), 512 * k:512 * k + 512],
                        ot[:],
                    )

    nc.compile()
    return nc


def _prep_inputs(fpsf, fimg, Wq, Wk):
    fpsf = np.ascontiguousarray(fpsf, dtype=np.float32)
    fimg = np.ascontiguousarray(fimg, dtype=np.float32)
    Wq = np.ascontiguousarray(Wq, dtype=np.float32)
    Wk = np.ascontiguousarray(Wk, dtype=np.float32)

    fpsfT = np.ascontiguousarray(fpsf.T)  # [64, 4]

    # WqT[r*64+e, q*128+m] = Wq[128*(2q+r)... careful: p = 2q + r? No:
    # rows r = p%2, cols q = p//2 with p the 128-chunk of hd.
    Wq4 = Wq.reshape(16, 2, 128, 64)  # [p//2, p%2, m, e]
    WqT = np.ascontiguousarray(
        Wq4.transpose(1, 3, 0, 2).reshape(128, 2048))  # [r*64+e, q*128+m]

    Wk3 = Wk.reshape(64, 64, 64)  # [h, d, c]
    bd = np.zeros((128, 32, 128), np.float32)
    bd[0:64, :, 0:64] = Wk3[0::2].transpose(1, 0, 2)   # [d, pair, c]
    bd[64:128, :, 64:128] = Wk3[1::2].transpose(1, 0, 2)
    Wk_bd = np.ascontiguousarray(bd.reshape(128, 4096))

    fimg_f = fimg.reshape(B, C, HW)
    in_maps = []
    for i in range(N_CORES):
        sh = np.ascontiguousarray(
            fimg_f[:, :, JS * i:JS * (i + 1)]).reshape(2 * 128, JS)
        in_maps.append({
            "fimg_s": sh,
            "fpsfT": fpsfT,
            "WqT": WqT,
            "Wk_bd": Wk_bd,
        })
    return in_maps


def kernel(fpsf, fimg, Wq, Wk):
    global _compiled
    if _compiled is None:
        _compiled = _build()
    nc = _compiled

    in_maps = _prep_inputs(fpsf, fimg, Wq, Wk)
    res = run_bass_kernel_spmd(nc, in_maps, core_ids=list(range(N_CORES)))

    out = np.empty((B, HEADS, HW), dtype=np.float32)
    for i in range(N_CORES):
        out[:, :, JS * i:JS * (i + 1)] = \
            res.results[i]["out"].reshape(B, HEADS, JS)
    return out.reshape(B, C, H, W)


if __name__ == "__main__":
    rng = np.random.default_rng(0)
    ins = {
        "fpsf": rng.standard_normal((B, C), dtype=np.float32),
        "fimg": rng.standard_normal((B, C, H, W), dtype=np.float32),
        "Wq": (rng.standard_normal((4096, C), dtype=np.float32) * 0.05),
        "Wk": (rng.standard_normal((4096, C), dtype=np.float32) * 0.05),
    }
    out = kernel(**ins)
    print("out", out.shape, out.dtype, float(np.abs(out).max()))
